# revision 11
# baseline (speedup 1.0000x reference)
"""Trainium2 Bass kernel for 3-layer CuGraphSAGE on a fanout-8 sampled tree.

The sampled graph produced by fanout-based neighbor sampling is a forest of
B=4096 independent trees (children of parent p are rows [4096+8p, 4096+8p+8)).
We shard by seed block: core c gets 512 seeds plus their full 3-hop subtrees
(4 contiguous row blocks of x, exactly 1/8 of all rows, zero halo).

Per-core pipeline (all activations channel-major [128ch, rows] so the matmul
contraction dim is always the partition dim — no transposes on device):
  mean-aggregation = 8 accumulating matmuls with stride-8 rhs APs, the 1/8
  folded into the aggregation weight; self term = 1 more matmul into the same
  PSUM bank; bias+ReLU on ScalarE evicts PSUM->SBUF. h1/h2 live entirely in
  SBUF; only x is streamed from HBM (153.6 MB/core) and 2.25 MB stored.
"""

import os
import numpy as np

# ---------------------------------------------------------------- constants
N_CORES = 8
C = 128                       # channels
B = 4096                      # seeds
S = B // N_CORES              # 512 seeds per core
BLK = [512, 4096, 32768, 262144]          # per-core rows per hop
OFF = [0, 4096, 36864, 299008]            # global start row of each hop block
NLOC = sum(BLK)                           # 299520 local rows
NPAR0 = BLK[0] + BLK[1] + BLK[2]          # 37376 local layer-0 parents
NPAR1 = BLK[0] + BLK[1]                   # 4608 local layer-1 parents
PT = 512                                  # parents per PSUM tile
N_FULL = 2396160
E_FULL = 2392064
OUT_ROWS = 36864

TRACE = os.environ.get("GNN_TRACE", "0") == "1"
DTYPE = os.environ.get("GNN_DTYPE", "float32")     # "float32" | "bfloat16"
LAST_RESULT = None

_BASS_CACHE = {}


def _build_bass(dtype_str):
    import concourse.mybir as mybir
    from concourse import bacc
    from concourse.tile import TileContext

    dt = getattr(mybir.dt, dtype_str)
    f32 = mybir.dt.float32
    Relu = mybir.ActivationFunctionType.Relu

    # Bacc (not raw Bass): its compile() pipeline splits multi-sem sync
    # waits into event semaphores — TRN2 allows at most 1 wait/instruction.
    nc = bacc.Bacc()
    xT = nc.dram_tensor("xT", [C, NLOC], dt, kind="ExternalInput")
    # all six 128x128 weight blocks packed into one tensor -> one DMA ->
    # one semaphore lane (per-instruction sync-wait slots are scarce)
    wconsts = nc.dram_tensor("wconsts", [C, 6 * C], dt, kind="ExternalInput")
    bconsts = nc.dram_tensor("bconsts", [C, 3], f32, kind="ExternalInput")
    out = nc.dram_tensor("out", [C, NPAR1], f32, kind="ExternalOutput")
    WIDX = {k: i for i, k in
            enumerate(("w1a", "w1b", "w2a", "w2b", "w3a", "w3b"))}

    with TileContext(nc) as tc:
        with tc.tile_pool(name="const", bufs=1) as constp, \
             tc.tile_pool(name="keep", bufs=1) as keepp, \
             tc.tile_pool(name="cbuf", bufs=2) as cpool, \
             tc.tile_pool(name="dbuf", bufs=3) as dpool, \
             tc.tile_pool(name="hbuf", bufs=2) as hpool, \
             tc.tile_pool(name="obuf", bufs=2) as opool, \
             tc.tile_pool(name="ps", bufs=6, space="PSUM") as pp:

            wtile = constp.tile([C, 6 * C], dt, name="wtile")
            nc.sync.dma_start(wtile[:, :], wconsts[:, :])
            btile = constp.tile([C, 3], f32, name="btile")
            nc.sync.dma_start(btile[:, :], bconsts[:, :])
            w = {k: wtile[:, C * i: C * (i + 1)] for k, i in WIDX.items()}
            bt = {f"b{i+1}": btile[:, i: i + 1] for i in range(3)}

            xA01 = keepp.tile([C, NPAR1], dt, tag="xA01")
            nc.sync.dma_start(xA01[:, :], xT[:, 0:NPAR1])
            h1self = keepp.tile([C, NPAR1], dt, tag="h1self")
            h2sb = keepp.tile([C, NPAR1], dt, tag="h2sb")

            def sage_tile(psum, wa, wb, children_ap, self_ap):
                # psum[o, p] = sum_e (W_a/8)[o,:] @ children[:, 8p+e]
                #            +  W_b[o,:] @ self[:, p]
                cv = children_ap.rearrange("c (p e) -> c p e", e=8)
                for e in range(8):
                    nc.tensor.matmul(psum, w[wa], cv[:, :, e],
                                     start=(e == 0), stop=False)
                nc.tensor.matmul(psum, w[wb], self_ap,
                                 start=False, stop=True)

            n_t = NPAR1 // PT                    # 9 outer tiles
            for t in range(n_t):
                # x rows [512+4096t, 512+4096(t+1)): children of layer-0
                # parents [512t, 512(t+1)) AND self-features of layer-0
                # parents [512+4096t, ...).
                Ct = cpool.tile([C, 8 * PT], dt, tag="C")
                nc.sync.dma_start(Ct[:, :],
                                  xT[:, S + 8 * PT * t: S + 8 * PT * (t + 1)])

                # layer-0 tile -> h1self[:, 512t:512(t+1)]
                ps0 = pp.tile([C, PT], f32, tag="ps")
                sage_tile(ps0, "w1a", "w1b", Ct[:, :],
                          xA01[:, PT * t: PT * (t + 1)])
                nc.scalar.activation(h1self[:, PT * t: PT * (t + 1)], ps0,
                                     Relu, bias=bt["b1"])

                # 8 layer-0 tiles for parents [512+4096t, 512+4096(t+1))
                h1tmp = hpool.tile([C, 8 * PT], dt, tag="h1tmp")
                for u in range(8):
                    base = NPAR1 + 8 * PT * (8 * t + u)
                    D = dpool.tile([C, 8 * PT], dt, tag="D")
                    nc.sync.dma_start(D[:, :], xT[:, base: base + 8 * PT])
                    psu = pp.tile([C, PT], f32, tag="ps")
                    sage_tile(psu, "w1a", "w1b", D[:, :],
                              Ct[:, PT * u: PT * (u + 1)])
                    nc.scalar.activation(h1tmp[:, PT * u: PT * (u + 1)], psu,
                                         Relu, bias=bt["b1"])

                # layer-1 tile for parents [512t, 512(t+1)) -> h2
                ps1 = pp.tile([C, PT], f32, tag="ps")
                sage_tile(ps1, "w2a", "w2b", h1tmp[:, :],
                          h1self[:, PT * t: PT * (t + 1)])
                nc.scalar.activation(h2sb[:, PT * t: PT * (t + 1)], ps1,
                                     Relu, bias=bt["b2"])

            # layer 2: parents [0, 512) aggregate h2[512:4608); rows
            # [512, 4608) have no in-edges (agg = 0) -> self term only.
            ps2 = pp.tile([C, PT], f32, tag="ps")
            sage_tile(ps2, "w3a", "w3b", h2sb[:, S:NPAR1], h2sb[:, 0:S])
            o0 = opool.tile([C, PT], f32, tag="o")
            nc.scalar.activation(o0[:, :], ps2, Relu, bias=bt["b3"])
            nc.sync.dma_start(out[:, 0:S], o0[:, :])
            for t in range(1, n_t):
                psn = pp.tile([C, PT], f32, tag="ps")
                nc.tensor.matmul(psn, w["w3b"],
                                 h2sb[:, PT * t: PT * (t + 1)],
                                 start=True, stop=True)
                on = opool.tile([C, PT], f32, tag="o")
                nc.scalar.activation(on[:, :], psn, Relu, bias=bt["b3"])
                nc.sync.dma_start(out[:, PT * t: PT * (t + 1)], on[:, :])

    nc.compile()
    return nc


def _get_bass(dtype_str):
    if dtype_str not in _BASS_CACHE:
        _BASS_CACHE[dtype_str] = _build_bass(dtype_str)
    return _BASS_CACHE[dtype_str]


def _edge_is_tree(edge):
    if edge.shape != (2, E_FULL):
        return False
    ar = np.arange(E_FULL, dtype=np.int64)
    return (np.array_equal(edge[0], (B + ar).astype(np.int32))
            and np.array_equal(edge[1], (ar // 8).astype(np.int32)))


def _fallback(x, edge, W1, b1, W2, b2, W3, b3):
    # General (structure-agnostic) CPU implementation; only used if the
    # inputs are not the fanout-8 tree this kernel is specialized for.
    sizes = [(N_FULL, E_FULL), (299008, 294912), (36864, 32768)]
    params = [(W1, b1), (W2, b2), (W3, b3)]
    x = x.astype(np.float32)
    for (n, e), (Wl, bl) in zip(sizes, params):
        src = edge[0, :e].astype(np.int64)
        dst = edge[1, :e].astype(np.int64)
        x = x[:n]
        agg = np.zeros((n, x.shape[1]), np.float32)
        np.add.at(agg, dst, x[src])
        deg = np.bincount(dst, minlength=n).astype(np.float32)
        agg /= np.maximum(deg, 1.0)[:, None]
        x = np.maximum(np.concatenate([agg, x], axis=1) @ Wl.T + bl, 0.0)
    return x


def kernel(**inputs):
    global LAST_RESULT
    x = np.asarray(inputs["x"])
    edge = np.asarray(inputs["edge"])
    W = [np.asarray(inputs[k], dtype=np.float32) for k in ("W1", "W2", "W3")]
    bias = [np.asarray(inputs[k], dtype=np.float32) for k in ("b1", "b2", "b3")]

    if x.shape != (N_FULL, C) or not _edge_is_tree(edge):
        return _fallback(x, edge, W[0], bias[0], W[1], bias[1], W[2], bias[2])

    from concourse.bass_utils import run_bass_kernel_spmd

    np_dt = {"float32": np.float32, "bfloat16": None}[DTYPE] or _bf16()
    x = np.ascontiguousarray(x, dtype=np.float32)

    wblocks = []
    for li in range(3):
        wblocks.append((W[li][:, :C] / 8.0).T)     # agg part, mean folded in
        wblocks.append(W[li][:, C:].T)             # self part
    wconsts = np.ascontiguousarray(np.concatenate(wblocks, axis=1)).astype(np_dt)
    bconsts = np.ascontiguousarray(np.stack(bias, axis=1))      # [128, 3] f32

    in_maps = []
    for c in range(N_CORES):
        xloc = np.concatenate(
            [x[OFF[h] + BLK[h] * c: OFF[h] + BLK[h] * (c + 1)] for h in range(4)],
            axis=0)
        xTc = np.ascontiguousarray(xloc.T).astype(np_dt, copy=False)
        in_maps.append({"xT": xTc, "wconsts": wconsts, "bconsts": bconsts})

    nc = _get_bass(DTYPE)
    res = run_bass_kernel_spmd(nc, in_maps, list(range(N_CORES)), trace=TRACE)
    LAST_RESULT = res

    out = np.empty((OUT_ROWS, C), np.float32)
    for c in range(N_CORES):
        oc = np.asarray(res.results[c]["out"])
        out[S * c: S * (c + 1)] = oc[:, :S].T
        out[B + 8 * S * c: B + 8 * S * (c + 1)] = oc[:, S:].T
    return out


def _bf16():
    import ml_dtypes
    return ml_dtypes.bfloat16


# revision 12
# speedup vs baseline: 1.9868x; 1.9868x over previous
"""Trainium2 Bass kernel for 3-layer CuGraphSAGE on a fanout-8 sampled tree.

The sampled graph produced by fanout-based neighbor sampling is a forest of
B=4096 independent trees (children of parent p are rows [4096+8p, 4096+8p+8)).
We shard by seed block: core c gets 512 seeds plus their full 3-hop subtrees
(4 contiguous row blocks of x, exactly 1/8 of all rows, zero halo).

Per-core pipeline (all activations channel-major [128ch, rows] so the matmul
contraction dim is always the partition dim — no transposes on device):
  mean-aggregation = 8 accumulating matmuls with stride-8 rhs APs, the 1/8
  folded into the aggregation weight; self term = 1 more matmul into the same
  PSUM bank; bias+ReLU on ScalarE evicts PSUM->SBUF. h1/h2 live entirely in
  SBUF; only x is streamed from HBM (153.6 MB/core) and 2.25 MB stored.
"""

import os
import numpy as np

# ---------------------------------------------------------------- constants
N_CORES = 8
C = 128                       # channels
B = 4096                      # seeds
S = B // N_CORES              # 512 seeds per core
BLK = [512, 4096, 32768, 262144]          # per-core rows per hop
OFF = [0, 4096, 36864, 299008]            # global start row of each hop block
NLOC = sum(BLK)                           # 299520 local rows
NPAR0 = BLK[0] + BLK[1] + BLK[2]          # 37376 local layer-0 parents
NPAR1 = BLK[0] + BLK[1]                   # 4608 local layer-1 parents
PT = 512                                  # parents per PSUM tile
N_FULL = 2396160
E_FULL = 2392064
OUT_ROWS = 36864

TRACE = os.environ.get("GNN_TRACE", "0") == "1"
DTYPE = os.environ.get("GNN_DTYPE", "float32")     # "float32" | "bfloat16"
LAST_RESULT = None

_BASS_CACHE = {}


def _build_bass(dtype_str):
    import concourse.mybir as mybir
    from concourse import bacc
    from concourse.tile import TileContext

    dt = getattr(mybir.dt, dtype_str)
    f32 = mybir.dt.float32
    Relu = mybir.ActivationFunctionType.Relu

    # Bacc (not raw Bass): its compile() pipeline splits multi-sem sync
    # waits into event semaphores — TRN2 allows at most 1 wait/instruction.
    nc = bacc.Bacc()
    xT = nc.dram_tensor("xT", [C, NLOC], dt, kind="ExternalInput")
    # all six 128x128 weight blocks packed into one tensor -> one DMA ->
    # one semaphore lane (per-instruction sync-wait slots are scarce)
    wconsts = nc.dram_tensor("wconsts", [C, 6 * C], dt, kind="ExternalInput")
    bconsts = nc.dram_tensor("bconsts", [C, 3], f32, kind="ExternalInput")
    out = nc.dram_tensor("out", [C, NPAR1], f32, kind="ExternalOutput")
    WIDX = {k: i for i, k in
            enumerate(("w1a", "w1b", "w2a", "w2b", "w3a", "w3b"))}

    with TileContext(nc) as tc:
        with tc.tile_pool(name="const", bufs=1) as constp, \
             tc.tile_pool(name="keep", bufs=1) as keepp, \
             tc.tile_pool(name="cbuf", bufs=2) as cpool, \
             tc.tile_pool(name="dbuf", bufs=3) as dpool, \
             tc.tile_pool(name="hbuf", bufs=2) as hpool, \
             tc.tile_pool(name="obuf", bufs=2) as opool, \
             tc.tile_pool(name="ps", bufs=6, space="PSUM") as pp:

            wtile = constp.tile([C, 6 * C], dt, name="wtile")
            nc.sync.dma_start(wtile[:, :], wconsts[:, :])
            btile = constp.tile([C, 3], f32, name="btile")
            nc.sync.dma_start(btile[:, :], bconsts[:, :])
            w = {k: wtile[:, C * i: C * (i + 1)] for k, i in WIDX.items()}
            bt = {f"b{i+1}": btile[:, i: i + 1] for i in range(3)}

            xA01 = keepp.tile([C, NPAR1], dt, tag="xA01")
            nc.sync.dma_start(xA01[:, :], xT[:, 0:NPAR1])
            h1self = keepp.tile([C, NPAR1], dt, tag="h1self")
            h2sb = keepp.tile([C, NPAR1], dt, tag="h2sb")

            def sage_tile(psum, wa, wb, children_ap, self_ap):
                # psum[o, p] = sum_e (W_a/8)[o,:] @ children[:, 8p+e]
                #            +  W_b[o,:] @ self[:, p]
                cv = children_ap.rearrange("c (p e) -> c p e", e=8)
                for e in range(8):
                    nc.tensor.matmul(psum, w[wa], cv[:, :, e],
                                     start=(e == 0), stop=False)
                nc.tensor.matmul(psum, w[wb], self_ap,
                                 start=False, stop=True)

            n_t = NPAR1 // PT                    # 9 outer tiles
            for t in range(n_t):
                # x rows [512+4096t, 512+4096(t+1)): children of layer-0
                # parents [512t, 512(t+1)) AND self-features of layer-0
                # parents [512+4096t, ...).
                Ct = cpool.tile([C, 8 * PT], dt, tag="C")
                nc.sync.dma_start(Ct[:, :],
                                  xT[:, S + 8 * PT * t: S + 8 * PT * (t + 1)])

                # layer-0 tile -> h1self[:, 512t:512(t+1)]
                ps0 = pp.tile([C, PT], f32, tag="ps")
                sage_tile(ps0, "w1a", "w1b", Ct[:, :],
                          xA01[:, PT * t: PT * (t + 1)])
                nc.scalar.activation(h1self[:, PT * t: PT * (t + 1)], ps0,
                                     Relu, bias=bt["b1"])

                # 8 layer-0 tiles for parents [512+4096t, 512+4096(t+1))
                h1tmp = hpool.tile([C, 8 * PT], dt, tag="h1tmp")
                for u in range(8):
                    base = NPAR1 + 8 * PT * (8 * t + u)
                    D = dpool.tile([C, 8 * PT], dt, tag="D")
                    nc.sync.dma_start(D[:, :], xT[:, base: base + 8 * PT])
                    psu = pp.tile([C, PT], f32, tag="ps")
                    sage_tile(psu, "w1a", "w1b", D[:, :],
                              Ct[:, PT * u: PT * (u + 1)])
                    nc.scalar.activation(h1tmp[:, PT * u: PT * (u + 1)], psu,
                                         Relu, bias=bt["b1"])

                # layer-1 tile for parents [512t, 512(t+1)) -> h2
                ps1 = pp.tile([C, PT], f32, tag="ps")
                sage_tile(ps1, "w2a", "w2b", h1tmp[:, :],
                          h1self[:, PT * t: PT * (t + 1)])
                nc.scalar.activation(h2sb[:, PT * t: PT * (t + 1)], ps1,
                                     Relu, bias=bt["b2"])

            # layer 2: parents [0, 512) aggregate h2[512:4608); rows
            # [512, 4608) have no in-edges (agg = 0) -> self term only.
            ps2 = pp.tile([C, PT], f32, tag="ps")
            sage_tile(ps2, "w3a", "w3b", h2sb[:, S:NPAR1], h2sb[:, 0:S])
            o0 = opool.tile([C, PT], f32, tag="o")
            nc.scalar.activation(o0[:, :], ps2, Relu, bias=bt["b3"])
            nc.sync.dma_start(out[:, 0:S], o0[:, :])
            for t in range(1, n_t):
                psn = pp.tile([C, PT], f32, tag="ps")
                nc.tensor.matmul(psn, w["w3b"],
                                 h2sb[:, PT * t: PT * (t + 1)],
                                 start=True, stop=True)
                on = opool.tile([C, PT], f32, tag="o")
                nc.scalar.activation(on[:, :], psn, Relu, bias=bt["b3"])
                nc.sync.dma_start(out[:, PT * t: PT * (t + 1)], on[:, :])

    nc.compile()
    return nc


def _get_bass(dtype_str):
    if dtype_str not in _BASS_CACHE:
        _BASS_CACHE[dtype_str] = _build_bass(dtype_str)
    return _BASS_CACHE[dtype_str]


def _edge_is_tree(edge):
    if edge.shape != (2, E_FULL):
        return False
    ar = np.arange(E_FULL, dtype=np.int64)
    return (np.array_equal(edge[0], (B + ar).astype(np.int32))
            and np.array_equal(edge[1], (ar // 8).astype(np.int32)))


def _fallback(x, edge, W1, b1, W2, b2, W3, b3):
    # General (structure-agnostic) CPU implementation; only used if the
    # inputs are not the fanout-8 tree this kernel is specialized for.
    sizes = [(N_FULL, E_FULL), (299008, 294912), (36864, 32768)]
    params = [(W1, b1), (W2, b2), (W3, b3)]
    x = x.astype(np.float32)
    for (n, e), (Wl, bl) in zip(sizes, params):
        src = edge[0, :e].astype(np.int64)
        dst = edge[1, :e].astype(np.int64)
        x = x[:n]
        agg = np.zeros((n, x.shape[1]), np.float32)
        np.add.at(agg, dst, x[src])
        deg = np.bincount(dst, minlength=n).astype(np.float32)
        agg /= np.maximum(deg, 1.0)[:, None]
        x = np.maximum(np.concatenate([agg, x], axis=1) @ Wl.T + bl, 0.0)
    return x


def kernel(**inputs):
    global LAST_RESULT
    x = np.asarray(inputs["x"])
    edge = np.asarray(inputs["edge"])
    W = [np.asarray(inputs[k], dtype=np.float32) for k in ("W1", "W2", "W3")]
    bias = [np.asarray(inputs[k], dtype=np.float32) for k in ("b1", "b2", "b3")]

    if x.shape != (N_FULL, C) or not _edge_is_tree(edge):
        return _fallback(x, edge, W[0], bias[0], W[1], bias[1], W[2], bias[2])

    from concourse.bass_utils import run_bass_kernel_spmd

    if DTYPE == "bfloat16":
        np_dt = _bf16()
    else:
        np_dt = {"float32": np.float32, "float16": np.float16}[DTYPE]
    x = np.ascontiguousarray(x, dtype=np.float32)

    wblocks = []
    for li in range(3):
        wblocks.append((W[li][:, :C] / 8.0).T)     # agg part, mean folded in
        wblocks.append(W[li][:, C:].T)             # self part
    wconsts = np.ascontiguousarray(np.concatenate(wblocks, axis=1)).astype(np_dt)
    bconsts = np.ascontiguousarray(np.stack(bias, axis=1))      # [128, 3] f32

    in_maps = []
    for c in range(N_CORES):
        xloc = np.concatenate(
            [x[OFF[h] + BLK[h] * c: OFF[h] + BLK[h] * (c + 1)] for h in range(4)],
            axis=0)
        xTc = np.ascontiguousarray(xloc.T).astype(np_dt, copy=False)
        in_maps.append({"xT": xTc, "wconsts": wconsts, "bconsts": bconsts})

    nc = _get_bass(DTYPE)
    res = run_bass_kernel_spmd(nc, in_maps, list(range(N_CORES)), trace=TRACE)
    LAST_RESULT = res

    out = np.empty((OUT_ROWS, C), np.float32)
    for c in range(N_CORES):
        oc = np.asarray(res.results[c]["out"])
        out[S * c: S * (c + 1)] = oc[:, :S].T
        out[B + 8 * S * c: B + 8 * S * (c + 1)] = oc[:, S:].T
    return out


def _bf16():
    import ml_dtypes
    return ml_dtypes.bfloat16


# revision 13
# speedup vs baseline: 2.5719x; 1.2945x over previous
"""Trainium2 Bass kernel for 3-layer CuGraphSAGE on a fanout-8 sampled tree.

The sampled graph produced by fanout-based neighbor sampling is a forest of
B=4096 independent trees (children of parent p are rows [4096+8p, 4096+8p+8)).
We shard by seed block: core c gets 512 seeds plus their full 3-hop subtrees
(4 contiguous row blocks of x, exactly 1/8 of all rows, zero halo).

Per-core pipeline (all activations channel-major [128ch, rows] so the matmul
contraction dim is always the partition dim — no transposes on device):
  mean-aggregation = 8 accumulating matmuls with stride-8 rhs APs, the 1/8
  folded into the aggregation weight; self term = 1 more matmul into the same
  PSUM bank; bias+ReLU on ScalarE evicts PSUM->SBUF. h1/h2 live entirely in
  SBUF; only x is streamed from HBM (153.6 MB/core) and 2.25 MB stored.
"""

import os
import numpy as np

# ---------------------------------------------------------------- constants
N_CORES = 8
C = 128                       # channels
B = 4096                      # seeds
S = B // N_CORES              # 512 seeds per core
BLK = [512, 4096, 32768, 262144]          # per-core rows per hop
OFF = [0, 4096, 36864, 299008]            # global start row of each hop block
NLOC = sum(BLK)                           # 299520 local rows
NPAR0 = BLK[0] + BLK[1] + BLK[2]          # 37376 local layer-0 parents
NPAR1 = BLK[0] + BLK[1]                   # 4608 local layer-1 parents
PT = 512                                  # parents per PSUM tile
N_FULL = 2396160
E_FULL = 2392064
OUT_ROWS = 36864

TRACE = os.environ.get("GNN_TRACE", "0") == "1"
DTYPE = os.environ.get("GNN_DTYPE", "float32")     # "float32" | "bfloat16"
LAST_RESULT = None

_BASS_CACHE = {}


def _build_bass(dtype_str):
    import concourse.mybir as mybir
    from concourse import bacc
    from concourse.tile import TileContext

    dt = getattr(mybir.dt, dtype_str)
    f32 = mybir.dt.float32
    Relu = mybir.ActivationFunctionType.Relu

    # Bacc (not raw Bass): its compile() pipeline splits multi-sem sync
    # waits into event semaphores — TRN2 allows at most 1 wait/instruction.
    nc = bacc.Bacc()
    xT = nc.dram_tensor("xT", [C, NLOC], dt, kind="ExternalInput")
    # all six 128x128 weight blocks packed into one tensor -> one DMA ->
    # one semaphore lane (per-instruction sync-wait slots are scarce)
    wconsts = nc.dram_tensor("wconsts", [C, 6 * C], dt, kind="ExternalInput")
    bconsts = nc.dram_tensor("bconsts", [C, 3], f32, kind="ExternalInput")
    out = nc.dram_tensor("out", [C, NPAR1], f32, kind="ExternalOutput")
    WIDX = {k: i for i, k in
            enumerate(("w1a", "w1b", "w2a", "w2b", "w3a", "w3b"))}

    with TileContext(nc) as tc:
        with tc.tile_pool(name="const", bufs=1) as constp, \
             tc.tile_pool(name="keep", bufs=1) as keepp, \
             tc.tile_pool(name="cbuf", bufs=2) as cpool, \
             tc.tile_pool(name="dbuf", bufs=3) as dpool, \
             tc.tile_pool(name="hbuf", bufs=2) as hpool, \
             tc.tile_pool(name="obuf", bufs=2) as opool, \
             tc.tile_pool(name="ps", bufs=6, space="PSUM") as pp:

            wtile = constp.tile([C, 6 * C], dt, name="wtile")
            nc.sync.dma_start(wtile[:, :], wconsts[:, :])
            btile = constp.tile([C, 3], f32, name="btile")
            nc.sync.dma_start(btile[:, :], bconsts[:, :])
            w = {k: wtile[:, C * i: C * (i + 1)] for k, i in WIDX.items()}
            bt = {f"b{i+1}": btile[:, i: i + 1] for i in range(3)}

            xA01 = keepp.tile([C, NPAR1], dt, tag="xA01")
            nc.sync.dma_start(xA01[:, :], xT[:, 0:NPAR1])
            h1self = keepp.tile([C, NPAR1], dt, tag="h1self")
            h2sb = keepp.tile([C, NPAR1], dt, tag="h2sb")

            def sage_tile(psum, wa, wb, children_ap, self_ap):
                # psum[o, p] = sum_e (W_a/8)[o,:] @ children[:, 8p+e]
                #            +  W_b[o,:] @ self[:, p]
                cv = children_ap.rearrange("c (p e) -> c p e", e=8)
                for e in range(8):
                    nc.tensor.matmul(psum, w[wa], cv[:, :, e],
                                     start=(e == 0), stop=False)
                nc.tensor.matmul(psum, w[wb], self_ap,
                                 start=False, stop=True)

            n_t = NPAR1 // PT                    # 9 outer tiles
            for t in range(n_t):
                # x rows [512+4096t, 512+4096(t+1)): children of layer-0
                # parents [512t, 512(t+1)) AND self-features of layer-0
                # parents [512+4096t, ...).
                Ct = cpool.tile([C, 8 * PT], dt, tag="C")
                nc.sync.dma_start(Ct[:, :],
                                  xT[:, S + 8 * PT * t: S + 8 * PT * (t + 1)])

                # layer-0 tile -> h1self[:, 512t:512(t+1)]
                ps0 = pp.tile([C, PT], f32, tag="ps")
                sage_tile(ps0, "w1a", "w1b", Ct[:, :],
                          xA01[:, PT * t: PT * (t + 1)])
                nc.scalar.activation(h1self[:, PT * t: PT * (t + 1)], ps0,
                                     Relu, bias=bt["b1"])

                # 8 layer-0 tiles for parents [512+4096t, 512+4096(t+1))
                h1tmp = hpool.tile([C, 8 * PT], dt, tag="h1tmp")
                for u in range(8):
                    base = NPAR1 + 8 * PT * (8 * t + u)
                    D = dpool.tile([C, 8 * PT], dt, tag="D")
                    nc.sync.dma_start(D[:, :], xT[:, base: base + 8 * PT])
                    psu = pp.tile([C, PT], f32, tag="ps")
                    sage_tile(psu, "w1a", "w1b", D[:, :],
                              Ct[:, PT * u: PT * (u + 1)])
                    nc.scalar.activation(h1tmp[:, PT * u: PT * (u + 1)], psu,
                                         Relu, bias=bt["b1"])

                # layer-1 tile for parents [512t, 512(t+1)) -> h2
                ps1 = pp.tile([C, PT], f32, tag="ps")
                sage_tile(ps1, "w2a", "w2b", h1tmp[:, :],
                          h1self[:, PT * t: PT * (t + 1)])
                nc.scalar.activation(h2sb[:, PT * t: PT * (t + 1)], ps1,
                                     Relu, bias=bt["b2"])

            # layer 2: parents [0, 512) aggregate h2[512:4608); rows
            # [512, 4608) have no in-edges (agg = 0) -> self term only.
            ps2 = pp.tile([C, PT], f32, tag="ps")
            sage_tile(ps2, "w3a", "w3b", h2sb[:, S:NPAR1], h2sb[:, 0:S])
            o0 = opool.tile([C, PT], f32, tag="o")
            nc.scalar.activation(o0[:, :], ps2, Relu, bias=bt["b3"])
            nc.sync.dma_start(out[:, 0:S], o0[:, :])
            for t in range(1, n_t):
                psn = pp.tile([C, PT], f32, tag="ps")
                nc.tensor.matmul(psn, w["w3b"],
                                 h2sb[:, PT * t: PT * (t + 1)],
                                 start=True, stop=True)
                on = opool.tile([C, PT], f32, tag="o")
                nc.scalar.activation(on[:, :], psn, Relu, bias=bt["b3"])
                nc.sync.dma_start(out[:, PT * t: PT * (t + 1)], on[:, :])

    nc.compile()
    return nc


def _get_bass(dtype_str):
    if dtype_str not in _BASS_CACHE:
        _BASS_CACHE[dtype_str] = _build_bass(dtype_str)
    return _BASS_CACHE[dtype_str]


def _edge_is_tree(edge):
    if edge.shape != (2, E_FULL):
        return False
    ar = np.arange(E_FULL, dtype=np.int64)
    return (np.array_equal(edge[0], (B + ar).astype(np.int32))
            and np.array_equal(edge[1], (ar // 8).astype(np.int32)))


def _fallback(x, edge, W1, b1, W2, b2, W3, b3):
    # General (structure-agnostic) CPU implementation; only used if the
    # inputs are not the fanout-8 tree this kernel is specialized for.
    sizes = [(N_FULL, E_FULL), (299008, 294912), (36864, 32768)]
    params = [(W1, b1), (W2, b2), (W3, b3)]
    x = x.astype(np.float32)
    for (n, e), (Wl, bl) in zip(sizes, params):
        src = edge[0, :e].astype(np.int64)
        dst = edge[1, :e].astype(np.int64)
        x = x[:n]
        agg = np.zeros((n, x.shape[1]), np.float32)
        np.add.at(agg, dst, x[src])
        deg = np.bincount(dst, minlength=n).astype(np.float32)
        agg /= np.maximum(deg, 1.0)[:, None]
        x = np.maximum(np.concatenate([agg, x], axis=1) @ Wl.T + bl, 0.0)
    return x


def kernel(**inputs):
    global LAST_RESULT
    x = np.asarray(inputs["x"])
    edge = np.asarray(inputs["edge"])
    W = [np.asarray(inputs[k], dtype=np.float32) for k in ("W1", "W2", "W3")]
    bias = [np.asarray(inputs[k], dtype=np.float32) for k in ("b1", "b2", "b3")]

    if x.shape != (N_FULL, C) or not _edge_is_tree(edge):
        return _fallback(x, edge, W[0], bias[0], W[1], bias[1], W[2], bias[2])

    from concourse.bass_utils import run_bass_kernel_spmd

    if DTYPE == "bfloat16":
        np_dt = _bf16()
    else:
        np_dt = {"float32": np.float32, "float32r": np.float32,
                 "float16": np.float16}[DTYPE]
    x = np.ascontiguousarray(x, dtype=np.float32)

    wblocks = []
    for li in range(3):
        wblocks.append((W[li][:, :C] / 8.0).T)     # agg part, mean folded in
        wblocks.append(W[li][:, C:].T)             # self part
    wconsts = np.ascontiguousarray(np.concatenate(wblocks, axis=1)).astype(np_dt)
    bconsts = np.ascontiguousarray(np.stack(bias, axis=1))      # [128, 3] f32

    in_maps = []
    for c in range(N_CORES):
        xloc = np.concatenate(
            [x[OFF[h] + BLK[h] * c: OFF[h] + BLK[h] * (c + 1)] for h in range(4)],
            axis=0)
        xTc = np.ascontiguousarray(xloc.T).astype(np_dt, copy=False)
        in_maps.append({"xT": xTc, "wconsts": wconsts, "bconsts": bconsts})

    nc = _get_bass(DTYPE)
    res = run_bass_kernel_spmd(nc, in_maps, list(range(N_CORES)), trace=TRACE)
    LAST_RESULT = res

    out = np.empty((OUT_ROWS, C), np.float32)
    for c in range(N_CORES):
        oc = np.asarray(res.results[c]["out"])
        out[S * c: S * (c + 1)] = oc[:, :S].T
        out[B + 8 * S * c: B + 8 * S * (c + 1)] = oc[:, S:].T
    return out


def _bf16():
    import ml_dtypes
    return ml_dtypes.bfloat16


# revision 19
# speedup vs baseline: 2.6485x; 1.0297x over previous
"""Trainium2 Bass kernel for 3-layer CuGraphSAGE on a fanout-8 sampled tree.

The sampled graph produced by fanout-based neighbor sampling is a forest of
B=4096 independent trees (children of parent p are rows [4096+8p, 4096+8p+8)).
We shard by seed block: core c gets 512 seeds plus their full 3-hop subtrees
(4 contiguous row blocks of x, exactly 1/8 of all rows, zero halo).

Per-core pipeline (all activations channel-major [128ch, rows] so the matmul
contraction dim is always the partition dim — no transposes on device):
  mean-aggregation = 8 accumulating matmuls with stride-8 rhs APs, the 1/8
  folded into the aggregation weight; self term = 1 more matmul into the same
  PSUM bank; bias+ReLU on ScalarE evicts PSUM->SBUF. h1/h2 live entirely in
  SBUF; only x is streamed from HBM (153.6 MB/core) and 2.25 MB stored.
"""

import os
import numpy as np

# ---------------------------------------------------------------- constants
N_CORES = 8
C = 128                       # channels
B = 4096                      # seeds
S = B // N_CORES              # 512 seeds per core
BLK = [512, 4096, 32768, 262144]          # per-core rows per hop
OFF = [0, 4096, 36864, 299008]            # global start row of each hop block
NLOC = sum(BLK)                           # 299520 local rows
NPAR0 = BLK[0] + BLK[1] + BLK[2]          # 37376 local layer-0 parents
NPAR1 = BLK[0] + BLK[1]                   # 4608 local layer-1 parents
PT = 512                                  # parents per PSUM tile
N_FULL = 2396160
E_FULL = 2392064
OUT_ROWS = 36864

TRACE = os.environ.get("GNN_TRACE", "0") == "1"
DTYPE = os.environ.get("GNN_DTYPE", "float32")
# aggregation path: "dve" = VectorE group-reduce + 1 matmul (best for f32,
# where matmul streams at 1/4 rate); "pe" = 8 accumulating matmuls with
# stride-8 rhs (best for 16-bit dtypes)
AGG = os.environ.get("GNN_AGG", "dve")
LAST_RESULT = None

_BASS_CACHE = {}


def _build_bass(dtype_str, agg):
    import concourse.mybir as mybir
    from concourse import bacc
    from concourse.tile import TileContext

    dt = getattr(mybir.dt, dtype_str)
    f32 = mybir.dt.float32
    Relu = mybir.ActivationFunctionType.Relu
    AxX = mybir.AxisListType.X

    # Bacc (not raw Bass): its compile() pipeline splits multi-sem sync
    # waits into event semaphores — TRN2 allows at most 1 wait/instruction.
    nc = bacc.Bacc()
    xT = nc.dram_tensor("xT", [C, NLOC], dt, kind="ExternalInput")
    # all six 128x128 weight blocks packed into one tensor -> one DMA ->
    # one semaphore lane (per-instruction sync-wait slots are scarce)
    wconsts = nc.dram_tensor("wconsts", [C, 6 * C], dt, kind="ExternalInput")
    bconsts = nc.dram_tensor("bconsts", [C, 3], f32, kind="ExternalInput")
    out = nc.dram_tensor("out", [C, NPAR1], f32, kind="ExternalOutput")
    WIDX = {k: i for i, k in
            enumerate(("w1a", "w1b", "w2a", "w2b", "w3a", "w3b"))}

    with TileContext(nc) as tc:
        with tc.tile_pool(name="const", bufs=1) as constp, \
             tc.tile_pool(name="keep", bufs=1) as keepp, \
             tc.tile_pool(name="cbuf", bufs=2) as cpool, \
             tc.tile_pool(name="dbuf", bufs=3) as dpool, \
             tc.tile_pool(name="hbuf", bufs=2) as hpool, \
             tc.tile_pool(name="obuf", bufs=2) as opool, \
             tc.tile_pool(name="aggbuf", bufs=4) as aggp, \
             tc.tile_pool(name="ps", bufs=6, space="PSUM") as pp:

            wtile = constp.tile([C, 6 * C], dt, name="wtile")
            nc.sync.dma_start(wtile[:, :], wconsts[:, :])
            btile = constp.tile([C, 3], f32, name="btile")
            nc.sync.dma_start(btile[:, :], bconsts[:, :])
            w = {k: wtile[:, C * i: C * (i + 1)] for k, i in WIDX.items()}
            bt = {f"b{i+1}": btile[:, i: i + 1] for i in range(3)}

            xA01 = keepp.tile([C, NPAR1], dt, tag="xA01")
            nc.sync.dma_start(xA01[:, :], xT[:, 0:NPAR1])
            h1self = keepp.tile([C, NPAR1], dt, tag="h1self")
            h2sb = keepp.tile([C, NPAR1], dt, tag="h2sb")

            def sage_tile(psum, wa, wb, children_ap, self_ap):
                # psum[o, p] = sum_e (W_a/8)[o,:] @ children[:, 8p+e]
                #            +  W_b[o,:] @ self[:, p]
                if agg == "pe":
                    cv = children_ap.rearrange("c (p e) -> c p e", e=8)
                    for e in range(8):
                        nc.tensor.matmul(psum, w[wa], cv[:, :, e],
                                         start=(e == 0), stop=False)
                else:
                    # group-sum the 8 siblings on VectorE (stride-1 inner
                    # reduce), then contract once on the TensorEngine --
                    # fp32 matmul streams at 1/4 rate, so 8x fewer matmuls
                    # wins even though DVE reduce is 1 elem/cycle/lane.
                    aggt = aggp.tile([C, PT], dt, tag="agg", name="aggt")
                    nc.vector.reduce_sum(
                        aggt[:, :],
                        children_ap.rearrange("c (p e) -> c p e", e=8),
                        axis=AxX)
                    nc.tensor.matmul(psum, w[wa], aggt[:, :],
                                     start=True, stop=False)
                nc.tensor.matmul(psum, w[wb], self_ap,
                                 start=False, stop=True)

            n_t = NPAR1 // PT                    # 9 outer tiles
            for t in range(n_t):
                # x rows [512+4096t, 512+4096(t+1)): children of layer-0
                # parents [512t, 512(t+1)) AND self-features of layer-0
                # parents [512+4096t, ...).
                Ct = cpool.tile([C, 8 * PT], dt, tag="C")
                nc.sync.dma_start(Ct[:, :],
                                  xT[:, S + 8 * PT * t: S + 8 * PT * (t + 1)])

                # layer-0 tile -> h1self[:, 512t:512(t+1)]
                ps0 = pp.tile([C, PT], f32, tag="ps")
                sage_tile(ps0, "w1a", "w1b", Ct[:, :],
                          xA01[:, PT * t: PT * (t + 1)])
                nc.scalar.activation(h1self[:, PT * t: PT * (t + 1)], ps0,
                                     Relu, bias=bt["b1"])

                # 8 layer-0 tiles for parents [512+4096t, 512+4096(t+1))
                h1tmp = hpool.tile([C, 8 * PT], dt, tag="h1tmp")
                for u in range(8):
                    base = NPAR1 + 8 * PT * (8 * t + u)
                    D = dpool.tile([C, 8 * PT], dt, tag="D")
                    nc.sync.dma_start(D[:, :], xT[:, base: base + 8 * PT])
                    psu = pp.tile([C, PT], f32, tag="ps")
                    sage_tile(psu, "w1a", "w1b", D[:, :],
                              Ct[:, PT * u: PT * (u + 1)])
                    nc.scalar.activation(h1tmp[:, PT * u: PT * (u + 1)], psu,
                                         Relu, bias=bt["b1"])

                # layer-1 tile for parents [512t, 512(t+1)) -> h2
                ps1 = pp.tile([C, PT], f32, tag="ps")
                sage_tile(ps1, "w2a", "w2b", h1tmp[:, :],
                          h1self[:, PT * t: PT * (t + 1)])
                nc.scalar.activation(h2sb[:, PT * t: PT * (t + 1)], ps1,
                                     Relu, bias=bt["b2"])

            # layer 2: parents [0, 512) aggregate h2[512:4608); rows
            # [512, 4608) have no in-edges (agg = 0) -> self term only.
            ps2 = pp.tile([C, PT], f32, tag="ps")
            sage_tile(ps2, "w3a", "w3b", h2sb[:, S:NPAR1], h2sb[:, 0:S])
            o0 = opool.tile([C, PT], f32, tag="o")
            nc.scalar.activation(o0[:, :], ps2, Relu, bias=bt["b3"])
            nc.sync.dma_start(out[:, 0:S], o0[:, :])
            for t in range(1, n_t):
                psn = pp.tile([C, PT], f32, tag="ps")
                nc.tensor.matmul(psn, w["w3b"],
                                 h2sb[:, PT * t: PT * (t + 1)],
                                 start=True, stop=True)
                on = opool.tile([C, PT], f32, tag="o")
                nc.scalar.activation(on[:, :], psn, Relu, bias=bt["b3"])
                nc.sync.dma_start(out[:, PT * t: PT * (t + 1)], on[:, :])

    nc.compile()
    return nc


def _get_bass(dtype_str, agg="dve"):
    key = (dtype_str, agg)
    if key not in _BASS_CACHE:
        _BASS_CACHE[key] = _build_bass(dtype_str, agg)
    return _BASS_CACHE[key]


def _edge_is_tree(edge):
    if edge.shape != (2, E_FULL):
        return False
    ar = np.arange(E_FULL, dtype=np.int64)
    return (np.array_equal(edge[0], (B + ar).astype(np.int32))
            and np.array_equal(edge[1], (ar // 8).astype(np.int32)))


def _fallback(x, edge, W1, b1, W2, b2, W3, b3):
    # General (structure-agnostic) CPU implementation; only used if the
    # inputs are not the fanout-8 tree this kernel is specialized for.
    sizes = [(N_FULL, E_FULL), (299008, 294912), (36864, 32768)]
    params = [(W1, b1), (W2, b2), (W3, b3)]
    x = x.astype(np.float32)
    for (n, e), (Wl, bl) in zip(sizes, params):
        src = edge[0, :e].astype(np.int64)
        dst = edge[1, :e].astype(np.int64)
        x = x[:n]
        agg = np.zeros((n, x.shape[1]), np.float32)
        np.add.at(agg, dst, x[src])
        deg = np.bincount(dst, minlength=n).astype(np.float32)
        agg /= np.maximum(deg, 1.0)[:, None]
        x = np.maximum(np.concatenate([agg, x], axis=1) @ Wl.T + bl, 0.0)
    return x


def kernel(**inputs):
    global LAST_RESULT
    x = np.asarray(inputs["x"])
    edge = np.asarray(inputs["edge"])
    W = [np.asarray(inputs[k], dtype=np.float32) for k in ("W1", "W2", "W3")]
    bias = [np.asarray(inputs[k], dtype=np.float32) for k in ("b1", "b2", "b3")]

    if x.shape != (N_FULL, C) or not _edge_is_tree(edge):
        return _fallback(x, edge, W[0], bias[0], W[1], bias[1], W[2], bias[2])

    from concourse.bass_utils import run_bass_kernel_spmd

    if DTYPE == "bfloat16":
        np_dt = _bf16()
    else:
        np_dt = {"float32": np.float32, "float32r": np.float32,
                 "float16": np.float16}[DTYPE]
    x = np.ascontiguousarray(x, dtype=np.float32)

    wblocks = []
    for li in range(3):
        wblocks.append((W[li][:, :C] / 8.0).T)     # agg part, mean folded in
        wblocks.append(W[li][:, C:].T)             # self part
    wconsts = np.ascontiguousarray(np.concatenate(wblocks, axis=1)).astype(np_dt)
    bconsts = np.ascontiguousarray(np.stack(bias, axis=1))      # [128, 3] f32

    in_maps = []
    for c in range(N_CORES):
        xloc = np.concatenate(
            [x[OFF[h] + BLK[h] * c: OFF[h] + BLK[h] * (c + 1)] for h in range(4)],
            axis=0)
        xTc = np.ascontiguousarray(xloc.T).astype(np_dt, copy=False)
        in_maps.append({"xT": xTc, "wconsts": wconsts, "bconsts": bconsts})

    nc = _get_bass(DTYPE, AGG)
    res = run_bass_kernel_spmd(nc, in_maps, list(range(N_CORES)), trace=TRACE)
    LAST_RESULT = res

    out = np.empty((OUT_ROWS, C), np.float32)
    for c in range(N_CORES):
        oc = np.asarray(res.results[c]["out"])
        out[S * c: S * (c + 1)] = oc[:, :S].T
        out[B + 8 * S * c: B + 8 * S * (c + 1)] = oc[:, S:].T
    return out


def _bf16():
    import ml_dtypes
    return ml_dtypes.bfloat16
